# revision 53
# baseline (speedup 1.0000x reference)
"""Causal multi-head attention (B=4, S=2048, H=16, D=64, E=1024) on 8 TRN2 cores.

Sharding: data-parallel over batch (4) x tensor-parallel over heads (2 groups
of 8). Each core computes, for its (batch, head-group):
    q/k/v projections -> causal softmax attention -> output projection
and returns a partial [S, E] output (Wr row-split); the host adds the two
partials per batch.

All matmul operands are bfloat16; PSUM accumulation stays fp32. Attention runs
in the transposed layout (keys/head-dims on partitions) so no on-chip
transposes are needed; V carries an extra ones-column so the attn@V matmul
also emits the softmax denominators (output row 64).

The PE executes its queue in order, so nothing slow may sit between matmuls:
softmax denominators are batched per query chunk (stream_shuffle moves each
head's denominator row to partitions 64..71, one reciprocal covers all 8
heads) and the reciprocal->broadcast chain stalls the PE once per chunk
instead of once per head.
"""

from contextlib import ExitStack

import numpy as np
import ml_dtypes

import concourse.bacc as bacc
import concourse.bass as bass
import concourse.mybir as mybir
import concourse.tile as tile
from concourse.bass_utils import run_bass_kernel_spmd

HEADS = 16
HD = 64
EMB = 1024
B, S = 4, 2048
SCALE = 1.0 / 8.0
NCORES = 8
HPC = HEADS // 2          # heads per core (8)
GW = HPC * HD             # head-group width (512)

F32 = mybir.dt.float32
BF16 = mybir.dt.bfloat16
EXP = mybir.ActivationFunctionType.Exp

NQC = 4                   # query chunks of 512
QW = 512                  # query chunk width
NKB = S // 128            # key blocks of 128 (16)
NEC = EMB // 128          # emb chunks (8)
NSB = S // 128            # seq blocks (16)


def build():
    nc = bacc.Bacc("TRN2", target_bir_lowering=False, debug=False)

    # x pre-swizzled on host to SBUF layout [p, ss, e, s] (slice-major)
    xt_d = nc.dram_tensor("xt", [128, 4, NEC, QW], BF16, kind="ExternalInput")
    # wq/wk pre-swizzled on host to [hp, p, e, n] so per-(hp) DMA is contiguous
    wq_d = nc.dram_tensor("wq", [4, 128, NEC, 128], BF16, kind="ExternalInput")
    wk_d = nc.dram_tensor("wk", [4, 128, NEC, 128], BF16, kind="ExternalInput")
    wv_d = nc.dram_tensor("wv", [EMB, GW], BF16, kind="ExternalInput")
    # wr pre-swizzled on host to SBUF layout [p, c, n]
    wr_d = nc.dram_tensor("wr", [128, 4, EMB], BF16, kind="ExternalInput")
    # consts: [:,0:512] shifted causal masks (cols 0:384 zero, 384:512 triu),
    #         [:,512:640] ones, [:,640:1152] per-head-pair selectors
    cn_d = nc.dram_tensor("consts", [128, 1152], BF16, kind="ExternalInput")
    y_d = nc.dram_tensor("y", [S, EMB], BF16, kind="ExternalOutput")

    with tile.TileContext(nc) as tc, nc.allow_low_precision(reason="bf16 attn"):
        with (
            tc.tile_pool(name="persist", bufs=1) as pp,
            tc.tile_pool(name="qtp", bufs=4) as pq,
            tc.tile_pool(name="outp", bufs=2) as po,
            tc.tile_pool(name="attn", bufs=4) as pa,
            tc.tile_pool(name="nump", bufs=2) as pn,
            tc.tile_pool(name="recp", bufs=2) as prc,
            tc.tile_pool(name="ysb", bufs=2) as pyb,
        ):
            kt = pp.tile([128, NQC, S], BF16, tag="kt")
            v = pp.tile([128, NKB, HPC, HD + 1], BF16, tag="v")
            wr = pp.tile([128, 4, EMB], BF16, tag="wr")
            mo = pp.tile([128, 1152], BF16, tag="consts")
            bigmask = mo[:, 0:512]
            selt = mo[0:8, 640:1152]

            # constants ride the ACT hardware DMA queue so the SP queue can
            # stream the compute-critical wv/x/wq/wk transfers unblocked
            nc.scalar.dma_start(mo[:], cn_d.ap())
            nc.scalar.dma_start(wr[:], wr_d.ap())
            # ones column of v (softmax denominator trick): engine copy, not
            # a strided DMA (byte-exact, so it can't race P1's v copies)
            nc.vector.tensor_copy(v[:, :, :, HD], mo[:, 512:640])

            qtiles = {}
            ctx3 = ExitStack()

            with (
                tc.tile_pool(name="proj", bufs=1) as pj,
                tc.tile_pool(name="wql", bufs=3) as pwq,
                tc.tile_pool(name="wkl", bufs=3) as pwk,
                tc.tile_pool(name="ps_mm", bufs=2, space="PSUM") as ps_mm,
            ):
                # slice-major so P1 can start after the first 1MB lands;
                # host pre-swizzle makes every DMA contiguous per partition
                xt4 = pj.tile([128, 4, NEC, QW], BF16, tag="xt")
                wv = pj.tile([128, NEC, GW], BF16, tag="wv")

                def wc_dma(dkey, c, hp):
                    pool, wsrc = (pwq, wq_d) if dkey == "q" else (pwk, wk_d)
                    wc = pool.tile([128, NEC, 128], BF16, tag="w",
                                   name=f"wc{dkey}{c}_{hp}")
                    nc.sync.dma_start(wc[:], wsrc.ap()[hp])
                    return wc

                # first PE group (P2 c=0) depends on just one wc + x slice 0;
                # queue order keeps every c=0 weight ahead of the bulk loads
                pre = {("q", 0, 0): wc_dma("q", 0, 0)}
                nc.sync.dma_start(xt4[:, 0, 0:4, :], xt_d.ap()[:, 0, 0:4, :])
                nc.sync.dma_start(xt4[:, 0, 4:8, :], xt_d.ap()[:, 0, 4:8, :])
                for dkey, hp in (("q", 1), ("q", 2), ("k", 0), ("k", 1),
                                 ("k", 2)):
                    pre[(dkey, 0, hp)] = wc_dma(dkey, 0, hp)
                nc.sync.dma_start(xt4[:, 1, :, :], xt_d.ap()[:, 1, :, :])
                pre[("q", 0, 3)] = wc_dma("q", 0, 3)
                pre[("k", 0, 3)] = wc_dma("k", 0, 3)
                for e in range(NEC):
                    nc.sync.dma_start(wv[:, e, :], wv_d.ap()[e * 128:(e + 1) * 128, :])
                for ss in range(2, 4):
                    nc.sync.dma_start(xt4[:, ss, :, :], xt_d.ap()[:, ss, :, :])

                def xt(e, lo, width):  # [128, width] x chunk at seq offset lo
                    ss, so = divmod(lo, QW)
                    return xt4[:, ss, e, so:so + width]

                # ---- P2: qT (per query chunk) and kT head-pair tiles ----
                for c in range(NQC):
                    qtile = pq.tile([128, NQC, QW], BF16, tag="qt")
                    qtiles[c] = qtile
                    for dst, dkey in ((qtile, "q"), (kt, "k")):
                        for hp in range(4):
                            wc = pre.pop((dkey, c, hp), None)
                            if wc is None:
                                wc = wc_dma(dkey, c, hp)
                            ps = ps_mm.tile([128, QW], F32, tag="mm")
                            for e in range(NEC):
                                nc.tensor.matmul(
                                    ps[:], wc[:, e, :], xt(e, c * QW, QW),
                                    start=(e == 0), stop=(e == NEC - 1),
                                )
                            if dst is qtile:
                                nc.any.tensor_copy(qtile[:, hp, :], ps[:])
                            else:
                                nc.any.tensor_copy(
                                    kt[:, hp, c * QW:(c + 1) * QW], ps[:])

                # ---- P1: v = x @ Wv, natural layout [seq, head, 64] ----
                for sb in range(NSB):
                    ps = ps_mm.tile([128, GW], F32, tag="mm")
                    for e in range(NEC):
                        nc.tensor.matmul(
                            ps[:], xt(e, sb * 128, 128), wv[:, e, :],
                            start=(e == 0), stop=(e == NEC - 1),
                        )
                    nc.vector.tensor_copy(
                        v[:, sb, :, 0:HD],
                        ps[:].rearrange("p (h d) -> p h d", d=HD),
                    )

            # ---- P3/P4: attention + output projection per query chunk.
            # Software-pipelined: chunk qc's normalize + P4 are issued after
            # chunk qc+1's attention, so the denominator DMA + reciprocal
            # latency hides behind the next chunk's matmul stream.
            ps3 = ctx3.enter_context(
                tc.tile_pool(name="ps_att", bufs=3, space="PSUM"))
            ps_out = ctx3.enter_context(
                tc.tile_pool(name="ps_out", bufs=2, space="PSUM"))
            ps_att = ps3
            state = {}

            def attn_phase(qc):
                kbmax = 4 * (qc + 1)
                qtile = qtiles[qc]
                num = pn.tile([128, 4, QW], BF16, tag="num", name=f"num{qc}")
                stage = prc.tile([1, HPC, QW], F32, tag="dstage",
                                 name=f"stage{qc}")
                last = qc == NQC - 1
                if last:
                    # final chunk: per-head eager reciprocals (partition 0 —
                    # approx_fast misbehaves at base partition 64) so the
                    # tail skips the cross-partition DMA + batched reciprocal
                    rech = prc.tile([1, HPC, QW], F32, tag="rech",
                                    name=f"rech{qc}", bufs=1)
                    recbh = prc.tile([1, HPC, QW], BF16, tag="recbh",
                                     name=f"recbh{qc}", bufs=1)
                else:
                    den = prc.tile([8, QW], F32, tag="den", name=f"den{qc}")
                # (h, g) stream with a one-group scores lookahead: scores for
                # item i+1 are issued before attnV of item i, so the PE never
                # sits behind the exp on the in-order queue
                items = [(h, g) for h in range(HPC) for g in range(kbmax // 2)]
                ats = {}
                outs = {}

                def issue_scores(idx):
                    h, g = items[idx]
                    hp, ho = h // 2, (h % 2) * HD
                    sc = ps_att.tile([128, 2, QW], F32, tag="sc",
                                     name=f"sc{qc}_{h}_{g}")
                    at = pa.tile([128, 2, QW], BF16, tag="at",
                                 name=f"at{qc}_{h}_{g}")
                    for s_ in range(2):
                        kb = 2 * g + s_
                        # scoresT block [keys, queries]
                        nc.tensor.matmul(
                            sc[:, s_, :],
                            kt[ho:ho + HD, hp, kb * 128:(kb + 1) * 128],
                            qtile[ho:ho + HD, hp, :],
                            start=True, stop=True,
                        )
                    nc.scalar.activation(at[:], sc[:], EXP)
                    for s_ in range(2):
                        kb = 2 * g + s_
                        j = kb - 4 * qc
                        if j >= 0:  # diagonal block: zero + tri in one mul
                            w = (j + 1) * 128
                            nc.vector.tensor_mul(
                                at[:, s_, 0:w],
                                at[:, s_, 0:w],
                                bigmask[:, 512 - w:512],
                            )
                    ats[idx] = at

                def issue_attnv(idx):
                    h, g = items[idx]
                    hp, ho = h // 2, (h % 2) * HD
                    if g == 0:
                        outs[h] = ps_out.tile([128, QW], F32, tag="out",
                                              name=f"o{qc}_{h}")
                    out_ps = outs[h]
                    at = ats.pop(idx)
                    for s_ in range(2):
                        kb = 2 * g + s_
                        nc.tensor.matmul(
                            out_ps[0:HD + 1, :],
                            v[:, kb, h, :],
                            at[:, s_, :],
                            start=(kb == 0), stop=(kb == kbmax - 1),
                        )
                    if 2 * g + 1 == kbmax - 1:
                        # rows 0..63 numerator, row 64 denom: stash numerator,
                        # park denom on p64 for the batched reciprocal
                        nc.vector.tensor_copy(num[ho:ho + HD, hp, :],
                                              out_ps[0:HD, :])
                        nc.scalar.copy(stage[0:1, h, :],
                                       out_ps[HD:HD + 1, :])
                        if last:
                            nc.vector.reciprocal_approx_fast(
                                rech[0:1, h, :], stage[0:1, h, :])
                            nc.scalar.copy(recbh[0:1, h, :],
                                           rech[0:1, h, :])

                issue_scores(0)
                if len(items) > 1:
                    issue_scores(1)
                for i in range(len(items)):
                    if i + 2 < len(items):
                        issue_scores(i + 2)
                    issue_attnv(i)
                # redistribute the 8 denom rows onto partitions 0..7, then
                # one reciprocal covers all 8 heads of this chunk
                if last:
                    state[qc] = (num, recbh, True)
                else:
                    nc.sync.dma_start(den[:, :], stage[0:1, :, :])
                    rec = prc.tile([8, QW], F32, tag="rec", name=f"rec{qc}")
                    nc.vector.reciprocal_approx_fast(rec[:], den[:])
                    recb = prc.tile([8, QW], BF16, tag="recb",
                                    name=f"recb{qc}")
                    nc.scalar.copy(recb[:], rec[:])
                    state[qc] = (num, recb, False)

            def norm_p4_phase(qc):
                num, recb, perhead = state.pop(qc)
                outtc = po.tile([128, NQC, QW], BF16, tag="outt",
                                name=f"outt{qc}")
                if perhead:
                    for h in range(HPC):
                        hp, ho = h // 2, (h % 2) * HD
                        bct = ps_out.tile([128, QW], F32, tag="out",
                                          name=f"bct{qc}_{h}")
                        nc.tensor.matmul(bct[0:HD, :],
                                         mo[0:1, 512:576],
                                         recb[0:1, h, :],
                                         start=True, stop=True)
                        nc.vector.tensor_mul(
                            outtc[ho:ho + HD, hp, :],
                            num[ho:ho + HD, hp, :], bct[0:HD, :],
                        )
                else:
                    for hp in range(4):
                        bct = ps_out.tile([128, QW], F32, tag="out",
                                          name=f"bct{qc}_{hp}")
                        nc.tensor.matmul(bct[:],
                                         selt[:, hp * 128:(hp + 1) * 128],
                                         recb[:],
                                         start=True, stop=True)
                        nc.vector.tensor_mul(
                            outtc[:, hp, :], num[:, hp, :], bct[:],
                        )
                # P4: y rows for this query chunk
                for sbl in range(4):
                    sb = qc * 4 + sbl
                    ysb = pyb.tile([128, EMB], BF16, tag="ysb",
                                   name=f"ysb{qc}_{sbl}")
                    for ncol in range(2):
                        ps = ps_out.tile([128, QW], F32, tag="out",
                                         name=f"y{qc}_{sbl}_{ncol}")
                        for hp in range(4):
                            nc.tensor.matmul(
                                ps[:],
                                outtc[:, hp, sbl * 128:(sbl + 1) * 128],
                                wr[:, hp, ncol * QW:(ncol + 1) * QW],
                                start=(hp == 0), stop=(hp == 3),
                            )
                        nc.any.tensor_copy(ysb[:, ncol * QW:(ncol + 1) * QW], ps[:])
                    nc.sync.dma_start(y_d.ap()[sb * 128:(sb + 1) * 128, :], ysb[:])

            for qc in range(NQC):
                attn_phase(qc)
                if qc >= 1:
                    norm_p4_phase(qc - 1)
            norm_p4_phase(NQC - 1)
            ctx3.close()

    nc.compile()
    return nc


_NC_CACHE = None


def _get_nc():
    global _NC_CACHE
    if _NC_CACHE is None:
        _NC_CACHE = build()
    return _NC_CACHE


def make_in_maps(x, Wq, Wk, Wv, Wr):
    BF = ml_dtypes.bfloat16
    x = np.ascontiguousarray(x, dtype=np.float32)
    Wq = np.asarray(Wq, dtype=np.float32)
    Wk = np.asarray(Wk, dtype=np.float32)
    Wv = np.asarray(Wv, dtype=np.float32)
    Wr = np.asarray(Wr, dtype=np.float32)

    consts = np.zeros((128, 1152), dtype=np.float32)
    # shifted causal mask: mask for diagonal offset j is consts[:, 512-(j+1)*128:512]
    consts[:, 384:512] = np.triu(np.ones((128, 128), dtype=np.float32))
    consts[:, 512:640] = 1.0
    # selector for head-pair hp: sel[p, hp*128 + m] = 1 iff p == 2*hp + (m >= 64)
    for hp in range(4):
        consts[2 * hp, 640 + hp * 128:640 + hp * 128 + 64] = 1.0
        consts[2 * hp + 1, 640 + hp * 128 + 64:640 + (hp + 1) * 128] = 1.0

    def swz(w):  # [1024, 512] -> [hp, p, e, n]
        return np.ascontiguousarray(
            w.reshape(NEC, 128, 4, 128).transpose(2, 1, 0, 3)).astype(BF)

    in_maps = []
    for core in range(NCORES):
        b, g = divmod(core, 2)
        hs = slice(g * GW, (g + 1) * GW)
        # [p, ss, e, s]: partition-major slice-major SBUF-exact layout
        xsw = x[b].T.reshape(NEC, 128, 4, QW).transpose(1, 2, 0, 3)
        in_maps.append({
            "xt": np.ascontiguousarray(xsw).astype(BF),
            "wq": swz(Wq[:, hs] * SCALE),
            "wk": swz(Wk[:, hs]),
            "wv": np.ascontiguousarray(Wv[:, hs]).astype(BF),
            "wr": np.ascontiguousarray(
                Wr[hs, :].reshape(4, 128, EMB).transpose(1, 0, 2)).astype(BF),
            "consts": consts.astype(BF),
        })
    return in_maps


def kernel(x, Wq, Wk, Wv, Wr):
    in_maps = make_in_maps(x, Wq, Wk, Wv, Wr)
    nc = _get_nc()
    res = run_bass_kernel_spmd(nc, in_maps, core_ids=list(range(NCORES)))

    y = np.empty((B, S, EMB), dtype=np.float32)
    for b in range(B):
        y[b] = (res.results[2 * b]["y"].astype(np.float32)
                + res.results[2 * b + 1]["y"].astype(np.float32))
    return y
